# revision 1
# baseline (speedup 1.0000x reference)
"""Trainium2 Bass kernel for nn_AnchorStores (retrieval_knn) — v2.

Per batch row b (one NeuronCore each, 8 cores data-parallel over B):
  dists[k] = mean_d qa[b,k,d] * (ln qa[b,k,d] - ln logits[b,d])   [K=256]
  top-8 over k, softmax, scatter into 4 classes by queue_label.

v2 removes every on-device transpose: the host stages qa pre-transposed
and in f16, laid out [128 partitions, NCH chunks, K] so partition p of
chunk c holds d = 128*c + p.  DMA reads are fully contiguous per
partition and HBM traffic halves vs f32.

Device pipeline per chunk group:
  - ACT: t = Ln(qa) on [128, G*K] f16 tiles (one big instr per group).
  - DVE: for a tunable fraction of chunks ("D-path"), fold the
    -ln(logits) subtraction into t via tensor_scalar (4x f16 mode);
    then one batched tensor_mul u = t*qa (2x f16 mode).
  - PE: ones^T @ u reduce over partitions (d) into psA [1, 2K]
    (even chunks -> cols 0..K, odd -> K..2K); for the remaining
    "P-path" chunks, (-ll_c)^T @ qa_c accumulates into psB [1, K].
  dists*D = psA_lo + psA_hi + psB.
Tiny epilogue: scale by -1/(T*D), DVE Max8, threshold mask + exp, and
two 1-column matmuls scatter softmax weights into the 4 classes via a
host-staged one-hot matrix.  Host divides the [1,4] output by its sum.
"""

import os
import sys

for _p in ("/opt/trn_rl_repo",):
    if _p not in sys.path:
        sys.path.insert(0, _p)

import numpy as np

B, K, DIM = 8, 256, 50257
KNN, N_CLASS = 8, 4
KNN_T = 0.05
CH = 128                       # d-chunk size (partition count)
NCH = (DIM + CH - 1) // CH     # 393 chunks
PAD_DIM = NCH * CH             # 50304
G_DMA = int(os.environ.get("KNN_G_DMA", "16"))   # chunks per DMA tile
G_ACT = int(os.environ.get("KNN_G_ACT", "8"))    # chunks per ACT/DVE group
DMA_SPLIT = int(os.environ.get("KNN_DMA_SPLIT", "2"))  # sub-DMAs per tile
# D-path (DVE subtract) fraction: chunks with (cg % DMOD) < DLIM
DMOD, DLIM = 16, int(os.environ.get("KNN_DLIM", "8"))
QA_BUFS = int(os.environ.get("KNN_QA_BUFS", "3"))
TU_BUFS = int(os.environ.get("KNN_TU_BUFS", "4"))


def _is_dpath(cg):
    return (cg % DMOD) < DLIM


def build_nc(repeat=1):
    import concourse.bass as bass
    import concourse.tile as tile
    from concourse import bacc, mybir

    F32 = mybir.dt.float32
    F16 = mybir.dt.float16
    AF = mybir.ActivationFunctionType
    ALU = mybir.AluOpType

    nc = bacc.Bacc("TRN2", target_bir_lowering=False, debug=False, num_devices=8)
    n_dt = (NCH + G_DMA - 1) // G_DMA
    # tile-blocked layout: rows [dt*128, (dt+1)*128) hold DMA tile dt as one
    # contiguous block, so every sub-DMA is a single linear DRAM stream.
    qa_d = nc.dram_tensor(
        "qa_t16", [n_dt * CH, G_DMA * K], F16, kind="ExternalInput")
    ll_d = nc.dram_tensor("ll_pi", [CH, NCH], F32, kind="ExternalInput")
    nll_d = nc.dram_tensor("negll16", [CH, NCH], F16, kind="ExternalInput")
    oh_d = nc.dram_tensor("lab_oh", [CH, 2 * N_CLASS], F32, kind="ExternalInput")
    out_d = nc.dram_tensor("out", [1, N_CLASS], F32, kind="ExternalOutput")

    n_dtiles = (NCH + G_DMA - 1) // G_DMA
    # chunk groups for ACT/DVE/PE, each within one DMA tile
    groups = []  # (first_chunk, n_chunks)
    c = 0
    while c < NCH:
        n = min(G_ACT, NCH - c)
        n = min(n, (c // G_DMA + 1) * G_DMA - c)
        groups.append((c, n))
        c += n
    total_pairs = sum((n + 1) // 2 for _, n in groups)
    n_ll = sum(
        1 for g0, gn in groups for ci in range(gn) if not _is_dpath(g0 + ci)
    )

    with tile.TileContext(nc) as tc:
        with (
            tc.tile_pool(name="consts", bufs=1) as consts,
            tc.tile_pool(name="qa", bufs=QA_BUFS) as qa_pool,
            tc.tile_pool(name="tu", bufs=TU_BUFS) as tu_pool,
            tc.tile_pool(name="eps", bufs=2) as eps_pool,
            tc.tile_pool(name="psd", bufs=2, space=bass.MemorySpace.PSUM) as psd_pool,
            tc.tile_pool(name="pse", bufs=2, space=bass.MemorySpace.PSUM) as pse_pool,
        ):
            ll_sb = consts.tile([CH, NCH], F32)
            nc.sync.dma_start(ll_sb[:], ll_d[:])
            nll_sb = consts.tile([CH, NCH], F16)
            nc.sync.dma_start(nll_sb[:], nll_d[:])
            oh_sb = consts.tile([CH, 2 * N_CLASS], F32)
            nc.sync.dma_start(oh_sb[:], oh_d[:])
            ones_sb = consts.tile([CH, 1], F32)
            nc.vector.memset(ones_sb[:], 1.0)
            ones16 = consts.tile([CH, 1], F16)
            nc.vector.memset(ones16[:], 1.0)

            def body():
                psA = psd_pool.tile([1, 2 * K], F32, tag="psA")
                if n_ll:
                    psB = psd_pool.tile([1, K], F32, tag="psB")
                pair_idx = 0
                ll_idx = 0

                for dt in range(n_dtiles):
                    c0 = dt * G_DMA
                    cw = min(G_DMA, NCH - c0)
                    qa_t = qa_pool.tile([CH, G_DMA * K], F16, tag="qa")
                    # split the tile DMA by partitions across queues
                    pstep = CH // DMA_SPLIT
                    for si in range(DMA_SPLIT):
                        p0 = si * pstep
                        nc.sync.dma_start(
                            qa_t[p0:p0 + pstep, 0:cw * K],
                            qa_d[dt * CH + p0:dt * CH + p0 + pstep, 0:cw * K],
                        )
                    for (g0, gn) in [g for g in groups if c0 <= g[0] < c0 + cw]:
                        off = (g0 - c0) * K
                        gw = gn * K
                        t_g = tu_pool.tile([CH, G_ACT * K], F16, tag="t")
                        nc.scalar.activation(
                            t_g[:, 0:gw], qa_t[:, off:off + gw], AF.Ln)
                        for ci in range(gn):
                            cg = g0 + ci
                            if _is_dpath(cg):
                                nc.vector.tensor_scalar(
                                    t_g[:, ci * K:(ci + 1) * K],
                                    t_g[:, ci * K:(ci + 1) * K],
                                    ll_sb[:, cg:cg + 1],
                                    None,
                                    op0=ALU.subtract,
                                )
                        u_g = tu_pool.tile([CH, G_ACT * K], F16, tag="u")
                        nc.vector.tensor_mul(
                            u_g[:, 0:gw], t_g[:, 0:gw], qa_t[:, off:off + gw])
                        # ll-MMs first: they depend only on the DMA (not on
                        # ACT/DVE), and grouping the ones-MMs afterwards
                        # loads the `ones` weight once per group.
                        for ci in range(gn):
                            cg = g0 + ci
                            if not _is_dpath(cg):
                                co = (g0 - c0 + ci) * K
                                nc.tensor.matmul(
                                    psB[0:1, 0:K],
                                    nll_sb[:, cg:cg + 1],
                                    qa_t[:, co:co + K],
                                    start=(ll_idx == 0),
                                    stop=(ll_idx == n_ll - 1),
                                    skip_group_check=True,
                                )
                                ll_idx += 1
                        for p0 in range(0, gn, 2):
                            pw = min(2, gn - p0) * K
                            nc.tensor.matmul(
                                psA[0:1, 0:pw],
                                ones16[:, 0:1],
                                u_g[:, p0 * K:p0 * K + pw],
                                start=(pair_idx == 0),
                                stop=(pair_idx == total_pairs - 1),
                                skip_group_check=True,
                            )
                            pair_idx += 1

                # ---- epilogue (tiny) ----
                h0 = eps_pool.tile([1, K], F32, tag="h0")
                nc.vector.tensor_copy(h0[:], psA[0:1, 0:K])
                h1 = eps_pool.tile([1, K], F32, tag="h1")
                nc.vector.tensor_add(h1[:], h0[:], psA[0:1, K:2 * K])
                if n_ll:
                    h2 = eps_pool.tile([1, K], F32, tag="h2")
                    nc.vector.tensor_add(h2[:], h1[:], psB[0:1, 0:K])
                else:
                    h2 = h1
                s_sb = eps_pool.tile([1, K], F32, tag="s")
                nc.vector.tensor_scalar_mul(s_sb[:], h2[:], -1.0 / (KNN_T * DIM))
                top8 = eps_pool.tile([1, 8], F32, tag="top8")
                nc.vector.max(top8[:], s_sb[:])
                negm = eps_pool.tile([1, 1], F32, tag="negm")
                nc.vector.tensor_scalar_mul(negm[:], top8[0:1, 0:1], -1.0)
                e_sb = eps_pool.tile([1, K], F32, tag="e")
                nc.scalar.activation(e_sb[:], s_sb[:], AF.Exp, bias=negm[0:1, 0:1])
                ge_sb = eps_pool.tile([1, K], F32, tag="ge")
                nc.vector.tensor_scalar(
                    ge_sb[:], s_sb[:], top8[0:1, 7:8], None, op0=ALU.is_ge)
                # unnormalized softmax weights; the host divides the [1,4]
                # output by its sum.
                w_sb = eps_pool.tile([1, K], F32, tag="w")
                nc.vector.tensor_mul(w_sb[:], e_sb[:], ge_sb[:])
                wcol_ps = pse_pool.tile([CH, 2], F32, tag="wcol")
                for g in (0, 1):
                    nc.tensor.matmul(
                        wcol_ps[:, g:g + 1],
                        w_sb[0:1, g * CH:(g + 1) * CH],
                        ones_sb[0:1, 0:1],
                        start=True, stop=True, skip_group_check=True)
                wcol_sb = eps_pool.tile([CH, 2], F32, tag="wcs")
                nc.vector.tensor_copy(wcol_sb[:], wcol_ps[:])
                prob_ps = pse_pool.tile([1, N_CLASS], F32, tag="prob")
                for g in (0, 1):
                    nc.tensor.matmul(
                        prob_ps[:],
                        wcol_sb[:, g:g + 1],
                        oh_sb[:, g * N_CLASS:(g + 1) * N_CLASS],
                        start=(g == 0), stop=(g == 1), skip_group_check=True)
                out_sb = eps_pool.tile([1, N_CLASS], F32, tag="osb")
                nc.vector.tensor_copy(out_sb[:], prob_ps[:])
                nc.sync.dma_start(out_d[:], out_sb[:])

            if repeat == 1:
                body()
            else:
                # unroll the repeat body so tile pools rotate across
                # consecutive iterations (cross-iteration overlap; the
                # For_i back-edge otherwise serializes fill/drain).
                un = max(1, int(os.environ.get("KNN_UNROLL", "2")))
                n_loop, tail = divmod(repeat, un)
                if n_loop:
                    with tc.For_i(0, n_loop, 1):
                        for _ in range(un):
                            body()
                for _ in range(tail):
                    body()

    nc.compile()
    return nc


def make_in_maps(logits, queue_anchor, queue_label):
    logits = np.asarray(logits, dtype=np.float32)
    qa = np.asarray(queue_anchor, dtype=np.float32)
    labels = np.asarray(queue_label)

    # [B, n_dt*128, G_DMA*K] f16, tile-blocked: row dt*128+p, col ci*K+k
    # holds qa[b, k, 128*(dt*G_DMA+ci) + p]
    n_dt = (NCH + G_DMA - 1) // G_DMA
    nch_pad = n_dt * G_DMA
    qa16 = np.ones((B, K, nch_pad * CH), np.float16)
    qa16[:, :, :DIM] = qa.astype(np.float16)
    qa_t16 = np.ascontiguousarray(
        qa16.reshape(B, K, n_dt, G_DMA, CH).transpose(0, 2, 4, 3, 1)
    ).reshape(B, n_dt * CH, G_DMA * K)

    ll_pad = np.zeros((B, PAD_DIM), np.float32)
    ll_pad[:, :DIM] = np.log(logits)
    ll_pi = np.ascontiguousarray(
        ll_pad.reshape(B, NCH, CH).transpose(0, 2, 1))  # [B, 128, NCH]
    negll16 = (-ll_pi).astype(np.float16)

    in_maps = []
    for b in range(B):
        oh = np.zeros((CH, 2 * N_CLASS), np.float32)
        lab = labels[b].astype(np.int64)
        for g in (0, 1):
            oh[np.arange(CH), N_CLASS * g + lab[g * CH:(g + 1) * CH]] = 1.0
        in_maps.append({
            "qa_t16": qa_t16[b],
            "ll_pi": ll_pi[b],
            "negll16": negll16[b],
            "lab_oh": oh,
        })
    return in_maps


_NC = None


def kernel(logits, queue_anchor, queue_label):
    global _NC
    from concourse.bass_utils import run_bass_kernel_spmd

    if _NC is None:
        _NC = build_nc(repeat=1)
    in_maps = make_in_maps(logits, queue_anchor, queue_label)
    # the time-shared device occasionally throws a transient
    # NRT_EXEC_UNIT_UNRECOVERABLE on dispatch; one retry recovers it
    import time

    for attempt in range(3):
        try:
            res = run_bass_kernel_spmd(_NC, in_maps, core_ids=list(range(B)))
            break
        except Exception:
            if attempt == 2:
                raise
            time.sleep(2.0)
    out = np.stack([np.asarray(res.results[i]["out"][0]) for i in range(B)])
    out = out / out.sum(axis=1, keepdims=True)
    return out.astype(np.float32)


if __name__ == "__main__":
    rng = np.random.default_rng(0)
    inputs = {
        "logits": rng.uniform(1e-3, 1.0, (B, DIM)).astype(np.float32),
        "queue_anchor": rng.uniform(1e-3, 1.0, (B, K, DIM)).astype(np.float32),
        "queue_label": rng.integers(0, N_CLASS, (B, K)).astype(np.int32),
    }
    out = kernel(**inputs)
    print(out)

